# revision 13
# baseline (speedup 1.0000x reference)
"""Trainium2 Bass kernel for the AMTCL loss (nn_AMTCL_66520453480770).

Math: the reference builds a [B,B] pairwise distance matrix dist[i,j] between
inputs[i] and centers[targets[j]] (weights 2**centers_weights[targets[j]]).
Since dist[i,j] depends on j only through c = targets[j], the whole problem
collapses to the [B,C] matrix
    dc2[i,c] = sum_d w2[c,d] * (centers[c,d] - inputs[i,d])**2
with
    dist_ap[i] = sqrt(dc2[i, t_i])            (all same-class j are equal)
    dist_an[i] = sqrt(min_{c present, c != t_i} dc2[i,c])   (sqrt commutes
                 with min, so mining happens on squared distances)
    cc[i]      = centers_dist[t_i],  centers_dist[c] = sqrt(min_{j!=c} cd2[c,j])
    loss       = mean(dist_ap + relu(cc - dist_an))
This is exactly equal to the reference (40x less compute than the B^2 form).

dc2 is one GEMM with contraction K = 2D+1 (bf16 inputs, fp32 PSUM):
    dc2[i,c] = sum_d xsq[i,d]*w2[c,d] + sum_d x[i,d]*(-2*w2[c,d]*c[c,d]) + a[c]
The a[c] row rides in as a K=1 term; the cd2 GEMM shares the same center-side
operands and adds 2^40*I via an identity matmul to mask the diagonal.

Sharding: data-parallel over the 4096 anchor rows -> 8 cores x 512 rows.
centers/centers_weights replicated. Each core emits a partial loss sum [1,1];
the host sums the 8 scalars and divides by B.
"""

import ml_dtypes
import numpy as np

import concourse.bass as bass
import concourse.bacc as bacc
import concourse.mybir as mybir
import concourse.tile as tile
from concourse.bass_utils import run_bass_kernel_spmd

B, C, D = 4096, 100, 384
NCORES = 8
ROWS = B // NCORES          # 512 anchor rows per core
MCH = ROWS // 128           # 4 partition chunks of anchor rows
KD = D // 128               # 3 partition chunks of the feature dim
PEN = float(2 ** 40)        # self-class / absent-class / diagonal penalty
LN2 = float(np.log(2.0))
F32 = mybir.dt.float32
BF16 = mybir.dt.bfloat16
AF = mybir.ActivationFunctionType
ALU = mybir.AluOpType

# f32 const block layout (columns): ones | eye100 | x | cT | cwT | penrow
ONES_O = 0
EYE_O = 128
XT_O = EYE_O + 100          # 228
CT_O = XT_O + KD * ROWS     # 228 + 1536 = 1764
CWT_O = CT_O + KD * C       # 2064
PROW_O = CWT_O + KD * C     # 2364 (row 0: PEN where class absent, else 0)
FW = PROW_O + C             # 2464

# bf16 const block layout: iota | t | eye | eyePEN | onesrow block
IOTA_O = 0
T_O = 100
BEYE_O = 104
BPEN_O = BEYE_O + 100       # 204
BONES_O = BPEN_O + 100      # 304
BW = BONES_O + 128          # 432


def build_nc() -> bass.Bass:
    nc = bacc.Bacc(
        "TRN2", target_bir_lowering=False, debug=False, num_devices=NCORES
    )

    fin = nc.declare_dram_parameter("fin", [128, FW], F32, isOutput=False)
    bin_ = nc.declare_dram_parameter("bin", [128, BW], BF16, isOutput=False)
    out = nc.declare_dram_parameter("out", [1, 1], F32, isOutput=True)

    with tile.TileContext(nc) as tc:
        with (
            tc.tile_pool(name="wts", bufs=1) as wp,
            tc.tile_pool(name="work", bufs=2) as kp,
            tc.tile_pool(name="ps1", bufs=1, space="PSUM") as pp1,
            tc.tile_pool(name="ps2", bufs=2, space="PSUM") as pp2,
        ):
            fsb = wp.tile([128, FW], F32, tag="fsb")
            nc.sync.dma_start(fsb[:], fin[:])
            bsb = wp.tile([128, BW], BF16, tag="bsb")
            nc.sync.dma_start(bsb[:], bin_[:])

            ones_f = fsb[:, ONES_O : ONES_O + 128]
            eye_f = fsb[:, EYE_O : EYE_O + 100]
            xt_f = fsb[:, XT_O : XT_O + KD * ROWS]
            ct_f = fsb[:, CT_O : CT_O + KD * C]
            cwt_f = fsb[:, CWT_O : CWT_O + KD * C]
            iota_b = bsb[:, IOTA_O : IOTA_O + 100]
            t_b = bsb[:, T_O : T_O + MCH]
            eye_b = bsb[:, BEYE_O : BEYE_O + 100]
            pen_b = bsb[:, BPEN_O : BPEN_O + 100]
            onesrow_b = bsb[0:1, BONES_O : BONES_O + 128]

            # ---- center-side prep ----
            w2f = wp.tile([128, KD * C], F32, tag="w2f")
            nc.scalar.activation(w2f[:], cwt_f, AF.Exp, scale=LN2)
            # dummy sqrt: loads the sqrt ACT table while PE/DMA are busy;
            # reads w2f so it is ordered after the Exp (exp table) use.
            sqdummy = wp.tile([1, 1], F32, tag="sqdummy")
            nc.scalar.sqrt(sqdummy[:], w2f[0:1, 0:1])

            csqf = wp.tile([128, KD * C], F32, tag="csqf")
            nc.scalar.square(csqf[:], ct_f)
            wsqf = wp.tile([128, KD * C], F32, tag="wsqf")
            nc.vector.tensor_tensor(wsqf[:], w2f[:], csqf[:], op=ALU.mult)
            w2cf = kp.tile([128, KD * C], F32, tag="w2cf")
            nc.gpsimd.tensor_tensor(w2cf[:], w2f[:], ct_f, op=ALU.mult)
            # bf16 casts of the GEMM operands
            m2b = wp.tile([128, KD * C], BF16, tag="m2b")
            nc.vector.tensor_scalar(m2b[:], w2cf[:], -2.0, None, op0=ALU.mult)
            w2b = wp.tile([128, KD * C], BF16, tag="w2b")
            nc.gpsimd.tensor_copy(w2b[:], w2f[:])
            cb = wp.tile([128, KD * C], BF16, tag="cb")
            nc.gpsimd.tensor_copy(cb[:], ct_f)
            csqb = wp.tile([128, KD * C], BF16, tag="csqb")
            nc.gpsimd.tensor_copy(csqb[:], csqf[:])

            # a[c] = sum_d w2*c^2 (fp32 matmuls), + absent-class pen, -> bf16
            psum_arow = pp1.tile([1, C], F32, tag="arow")
            for k in range(KD):
                nc.tensor.matmul(
                    psum_arow[:], ones_f[:, 0:1],
                    wsqf[:, k * C : (k + 1) * C],
                    start=(k == 0), stop=(k == KD - 1),
                )
            arowb = wp.tile([1, C], BF16, tag="arowb")
            nc.vector.tensor_tensor(
                arowb[:], psum_arow[:], fsb[0:1, PROW_O : PROW_O + C],
                op=ALU.add,
            )

            # ---- anchor-side prep: bf16 x and x^2 ----
            xb = wp.tile([128, KD * ROWS], BF16, tag="xb")
            nc.vector.tensor_copy(xb[:], xt_f)
            xsqb = wp.tile([128, KD * ROWS], BF16, tag="xsqb")
            nc.scalar.square(xsqb[:], xt_f)

            # ---- cd2 GEMM [100,100]: same center operands + PEN*I diag ----
            psum_cd2 = pp1.tile([C, C], F32, tag="cd2")
            for k in range(KD):
                nc.tensor.matmul(
                    psum_cd2[:], m2b[:, k * C : (k + 1) * C],
                    cb[:, k * C : (k + 1) * C],
                    start=(k == 0), stop=False,
                )
                nc.tensor.matmul(
                    psum_cd2[:], w2b[:, k * C : (k + 1) * C],
                    csqb[:, k * C : (k + 1) * C],
                    start=False, stop=False,
                )
            nc.tensor.matmul(
                psum_cd2[:], pen_b[0:100, :], eye_b[0:100, :],
                start=False, stop=False,
            )
            nc.tensor.matmul(
                psum_cd2[:], arowb[:], onesrow_b[:, 0:C],
                start=False, stop=True,
            )
            cd2m = wp.tile([C, C], F32, tag="cd2m")
            nc.vector.tensor_scalar(cd2m[:], psum_cd2[:], 0.0, None, op0=ALU.max)
            cdmin2 = wp.tile([C, 1], F32, tag="cdmin2")
            nc.vector.tensor_reduce(
                cdmin2[:], cd2m[:], axis=mybir.AxisListType.X, op=ALU.min
            )
            cdmin = wp.tile([C, 1], F32, tag="cdmin")
            nc.scalar.sqrt(cdmin[:], cdmin2[:])
            # transpose to a row, then broadcast down 128 partitions (fp32)
            psum_cdrow = pp1.tile([1, C], F32, tag="cdrow")
            nc.tensor.matmul(psum_cdrow[:], cdmin[:], eye_f[0:100, :])
            cdrow = wp.tile([1, C], F32, tag="cdrow_sb")
            nc.scalar.copy(cdrow[:], psum_cdrow[:])
            psum_bc = pp1.tile([128, C], F32, tag="bcast")
            nc.tensor.matmul(psum_bc[:], ones_f[0:1, :], cdrow[:])
            cdbb = wp.tile([128, C], BF16, tag="cdbb")
            nc.scalar.copy(cdbb[:], psum_bc[:])

            # ---- big GEMM: dc2 per 128-anchor chunk (bf16, fp32 PSUM) ----
            dc2w = wp.tile([128, MCH * C], BF16, tag="dc2w")
            for m in range(MCH):
                sl = slice(m * 128, (m + 1) * 128)
                psum_dc2 = pp2.tile([128, C], F32, tag="dc2")
                for k in range(KD):
                    ksl = slice(k * ROWS + m * 128, k * ROWS + (m + 1) * 128)
                    nc.tensor.matmul(
                        psum_dc2[:], xsqb[:, ksl],
                        w2b[:, k * C : (k + 1) * C],
                        start=(k == 0), stop=False,
                    )
                    nc.tensor.matmul(
                        psum_dc2[:], xb[:, ksl],
                        m2b[:, k * C : (k + 1) * C],
                        start=False, stop=False,
                    )
                nc.tensor.matmul(
                    psum_dc2[:], onesrow_b[:], arowb[:],
                    start=False, stop=True,
                )
                nc.scalar.copy(dc2w[:, m * C : (m + 1) * C], psum_dc2[:])

            # ---- batched mining over [128, MCH, C] bf16 views ----
            dc3 = dc2w[:].rearrange("p (m c) -> p m c", c=C)
            iota3 = iota_b[:, None, :].broadcast_to([128, MCH, C])
            t3 = t_b[:, :, None].broadcast_to([128, MCH, C])
            cdb3 = cdbb[:, None, :].broadcast_to([128, MCH, C])

            ohw = kp.tile([128, MCH * C], BF16, tag="ohw")
            oh3 = ohw[:].rearrange("p (m c) -> p m c", c=C)
            nc.vector.tensor_tensor(oh3, iota3, t3, op=ALU.is_equal)
            ohpw = kp.tile([128, MCH * C], BF16, tag="ohpw")
            nc.gpsimd.tensor_scalar(ohpw[:], ohw[:], PEN, None, op0=ALU.mult)
            antw = kp.tile([128, MCH * C], BF16, tag="antw")
            nc.gpsimd.tensor_tensor(
                antw[:].rearrange("p (m c) -> p m c", c=C), dc3,
                ohpw[:].rearrange("p (m c) -> p m c", c=C), op=ALU.add
            )
            an2 = kp.tile([128, MCH], F32, tag="an2")
            nc.vector.tensor_reduce(
                an2[:], antw[:].rearrange("p (m c) -> p m c", c=C),
                axis=mybir.AxisListType.X, op=ALU.min,
            )
            aptw = kp.tile([128, MCH * C], BF16, tag="aptw")
            nc.vector.tensor_tensor(
                aptw[:].rearrange("p (m c) -> p m c", c=C), dc3, oh3,
                op=ALU.mult,
            )
            ap2 = kp.tile([128, MCH], F32, tag="ap2")
            nc.vector.tensor_reduce(
                ap2[:], aptw[:].rearrange("p (m c) -> p m c", c=C),
                axis=mybir.AxisListType.X, op=ALU.add,
            )
            cctw = kp.tile([128, MCH * C], BF16, tag="cctw")
            nc.vector.tensor_tensor(
                cctw[:].rearrange("p (m c) -> p m c", c=C), cdb3, oh3,
                op=ALU.mult,
            )
            cc4 = kp.tile([128, MCH], F32, tag="cc4")
            nc.vector.tensor_reduce(
                cc4[:], cctw[:].rearrange("p (m c) -> p m c", c=C),
                axis=mybir.AxisListType.X, op=ALU.add,
            )

            # ---- loss_i = sqrt(ap2) + relu(cc - sqrt(an2)) ----
            an = kp.tile([128, MCH], F32, tag="an")
            nc.scalar.sqrt(an[:], an2[:])
            ap = kp.tile([128, MCH], F32, tag="ap")
            nc.scalar.sqrt(ap[:], ap2[:])
            mrgin = kp.tile([128, MCH], F32, tag="mrgin")
            nc.vector.tensor_tensor(mrgin[:], cc4[:], an[:], op=ALU.subtract)
            mrg = kp.tile([128, MCH], F32, tag="mrg")
            nc.scalar.activation(mrg[:], mrgin[:], AF.Relu)
            loss4 = kp.tile([128, MCH], F32, tag="loss4")
            nc.vector.tensor_tensor(loss4[:], mrg[:], ap[:], op=ALU.add)
            losscol = kp.tile([128, 1], F32, tag="losscol")
            nc.vector.tensor_reduce(
                losscol[:], loss4[:], axis=mybir.AxisListType.X, op=ALU.add
            )

            psum_loss = pp1.tile([1, 1], F32, tag="loss")
            nc.tensor.matmul(psum_loss[:], ones_f[:, 0:1], losscol[:])
            res_sb = wp.tile([1, 1], F32, tag="res")
            nc.scalar.copy(res_sb[:], psum_loss[:])
            nc.sync.dma_start(out[:], res_sb[:])

    nc.compile()
    return nc


_NC_CACHE: list = []


def _get_nc() -> bass.Bass:
    if not _NC_CACHE:
        _NC_CACHE.append(build_nc())
    return _NC_CACHE[0]


def make_in_maps(inputs, centers, centers_weights, targets):
    x = np.asarray(inputs, dtype=np.float32)
    c = np.asarray(centers, dtype=np.float32)
    cw = np.asarray(centers_weights, dtype=np.float32)
    t = np.asarray(targets).astype(np.int64)

    # bf16 const block (replicated)
    bconst = np.zeros((128, BW), dtype=np.float32)
    bconst[:, IOTA_O : IOTA_O + 100] = np.arange(C, dtype=np.float32)[None, :]
    eye = np.eye(C, dtype=np.float32)
    bconst[0:100, BEYE_O : BEYE_O + 100] = eye
    bconst[0:100, BPEN_O : BPEN_O + 100] = PEN * eye
    bconst[0:1, BONES_O : BONES_O + 128] = 1.0

    # f32 const block: shared part
    fshared = np.zeros((128, FW), dtype=np.float32)
    fshared[:, ONES_O : ONES_O + 128] = 1.0
    fshared[0:100, EYE_O : EYE_O + 100] = eye
    present = np.zeros(C, dtype=bool)
    present[np.unique(t)] = True
    fshared[0, PROW_O : PROW_O + C] = np.where(present, 0.0, PEN)
    cT = c.T.reshape(KD, 128, C)
    for k in range(KD):
        fshared[:, CT_O + k * C : CT_O + (k + 1) * C] = cT[k]
    cwT = cw.T.reshape(KD, 128, C)
    for k in range(KD):
        fshared[:, CWT_O + k * C : CWT_O + (k + 1) * C] = cwT[k]

    xT = np.ascontiguousarray(x.T)                      # [D, B]

    in_maps = []
    for i in range(NCORES):
        rows = slice(i * ROWS, (i + 1) * ROWS)
        f = fshared.copy()
        xs = xT[:, rows].reshape(KD, 128, ROWS)
        for k in range(KD):
            f[:, XT_O + k * ROWS : XT_O + (k + 1) * ROWS] = xs[k]
        bcst = bconst.copy()
        ts = t[rows].astype(np.float32).reshape(MCH, 128)
        bcst[:, T_O : T_O + MCH] = ts.T
        in_maps.append({
            "fin": f,
            "bin": bcst.astype(ml_dtypes.bfloat16),
        })
    return in_maps


def kernel(inputs, centers, centers_weights, targets, epoch_number=None,
           **_ignored):
    nc = _get_nc()
    in_maps = make_in_maps(inputs, centers, centers_weights, targets)
    res = run_bass_kernel_spmd(nc, in_maps, core_ids=list(range(NCORES)))
    total = sum(float(r["out"][0, 0]) for r in res.results)
    return np.float32(total / B)


# revision 14
# speedup vs baseline: 1.5037x; 1.5037x over previous
"""Trainium2 Bass kernel for the AMTCL loss (nn_AMTCL_66520453480770).

Math: the reference builds a [B,B] pairwise distance matrix dist[i,j] between
inputs[i] and centers[targets[j]] (weights 2**centers_weights[targets[j]]).
Since dist[i,j] depends on j only through c = targets[j], the whole problem
collapses to the [B,C] matrix
    dc2[i,c] = sum_d w2[c,d] * (centers[c,d] - inputs[i,d])**2
with
    dist_ap[i] = sqrt(dc2[i, t_i])            (all same-class j are equal)
    dist_an[i] = sqrt(min_{c present, c != t_i} dc2[i,c])   (sqrt commutes
                 with min, so mining happens on squared distances)
    cc[i]      = centers_dist[t_i],  centers_dist[c] = sqrt(min_{j!=c} cd2[c,j])
    loss       = mean(dist_ap + relu(cc - dist_an))
This is exactly equal to the reference (40x less compute than the B^2 form);
compute runs in bf16 with fp32 PSUM accumulation (loss rel err ~1e-4).

dc2 is one GEMM with contraction K = 2D+1:
    dc2[i,c] = sum_d xsq[i,d]*w2[c,d] + sum_d x[i,d]*(-2*w2[c,d]*c[c,d]) + a[c]
The a[c] row rides in as a K=1 term; the cd2 GEMM shares the same center-side
operands and adds 2^40*I via an identity matmul to mask the diagonal.

Sharding: data-parallel over the 4096 anchor rows -> 8 cores x 512 rows.
centers/centers_weights replicated. Each core emits a partial loss sum [1,1];
the host sums the 8 scalars and divides by B.
"""

import ml_dtypes
import numpy as np

import concourse.bass as bass
import concourse.bacc as bacc
import concourse.mybir as mybir
import concourse.tile as tile
from concourse.bass_utils import run_bass_kernel_spmd

B, C, D = 4096, 100, 384
NCORES = 8
ROWS = B // NCORES          # 512 anchor rows per core
MCH = ROWS // 128           # 4 partition chunks of anchor rows
KD = D // 128               # 3 partition chunks of the feature dim
PEN = float(2 ** 40)        # self-class / absent-class / diagonal penalty
LN2 = float(np.log(2.0))
F32 = mybir.dt.float32
BF16 = mybir.dt.bfloat16
AF = mybir.ActivationFunctionType
ALU = mybir.AluOpType

# bf16 const block layout (columns)
IOTA_O = 0                   # iota row 0..99, all partitions
T_O = IOTA_O + 100           # targets column-chunks [128, MCH]
BEYE_O = T_O + MCH           # eye(100)
BPEN_O = BEYE_O + 100        # PEN * eye(100)
BONES_O = BPEN_O + 100       # all-ones [128,128]
CT_O = BONES_O + 128         # centers.T chunks [128, 3*100]
CWT_O = CT_O + KD * C        # centers_weights.T chunks
XT_O = CWT_O + KD * C        # x.T shard chunks [128, 3*512]
BW = XT_O + KD * ROWS        # 2568

# f32 const block: ones column | absent-class penalty row (row 0)
FPEN_O = 1
FW = FPEN_O + C


def build_nc() -> bass.Bass:
    nc = bacc.Bacc(
        "TRN2", target_bir_lowering=False, debug=False, num_devices=NCORES
    )

    bin_ = nc.declare_dram_parameter("bin", [128, BW], BF16, isOutput=False)
    fin = nc.declare_dram_parameter("fin", [128, FW], F32, isOutput=False)
    out = nc.declare_dram_parameter("out", [1, 1], F32, isOutput=True)

    with tile.TileContext(nc) as tc:
        with (
            tc.tile_pool(name="wts", bufs=1) as wp,
            tc.tile_pool(name="work", bufs=2) as kp,
            tc.tile_pool(name="ps1", bufs=1, space="PSUM") as pp1,
            tc.tile_pool(name="ps2", bufs=2, space="PSUM") as pp2,
        ):
            bsb = wp.tile([128, BW], BF16, tag="bsb")
            nc.sync.dma_start(bsb[:], bin_[:])
            fsb = wp.tile([128, FW], F32, tag="fsb")
            nc.sync.dma_start(fsb[:], fin[:])

            iota_b = bsb[:, IOTA_O : IOTA_O + 100]
            t_b = bsb[:, T_O : T_O + MCH]
            eye_b = bsb[:, BEYE_O : BEYE_O + 100]
            eyepen_b = bsb[:, BPEN_O : BPEN_O + 100]
            ones_b = bsb[:, BONES_O : BONES_O + 128]
            ct_b = bsb[:, CT_O : CT_O + KD * C]
            cwt_b = bsb[:, CWT_O : CWT_O + KD * C]
            xt_b = bsb[:, XT_O : XT_O + KD * ROWS]
            ones_f = fsb[:, 0:1]
            penrow_f = fsb[0:1, FPEN_O : FPEN_O + C]

            # ---- center-side prep (all bf16, fast engines) ----
            w2b = wp.tile([128, KD * C], BF16, tag="w2b")
            nc.scalar.activation(w2b[:], cwt_b, AF.Exp, scale=LN2)
            # dummy sqrt: pulls the sqrt ACT table load off the critical
            # path; reads w2b so it lands after the Exp (exp-table) use.
            sqdummy = wp.tile([1, 1], F32, tag="sqdummy")
            nc.scalar.sqrt(sqdummy[:], w2b[0:1, 0:1])

            csqb = wp.tile([128, KD * C], BF16, tag="csqb")
            nc.vector.tensor_tensor(csqb[:], ct_b, ct_b, op=ALU.mult)
            cm2b = wp.tile([128, KD * C], BF16, tag="cm2b")
            nc.vector.tensor_scalar(cm2b[:], ct_b, -2.0, None, op0=ALU.mult)
            m2b = wp.tile([128, KD * C], BF16, tag="m2b")
            nc.vector.tensor_tensor(m2b[:], w2b[:], cm2b[:], op=ALU.mult)
            wsqb = wp.tile([128, KD * C], BF16, tag="wsqb")
            nc.vector.tensor_tensor(wsqb[:], w2b[:], csqb[:], op=ALU.mult)

            # a[c] = sum_d w2*c^2, + absent-class penalty row -> bf16
            psum_arow = pp1.tile([1, C], F32, tag="arow")
            for k in range(KD):
                nc.tensor.matmul(
                    psum_arow[:], ones_b[:, 0:1],
                    wsqb[:, k * C : (k + 1) * C],
                    start=(k == 0), stop=(k == KD - 1),
                )
            arowb = wp.tile([1, C], BF16, tag="arowb")
            nc.vector.tensor_tensor(
                arowb[:], psum_arow[:], penrow_f, op=ALU.add
            )

            # ---- anchor-side prep ----
            xsqb = wp.tile([128, KD * ROWS], BF16, tag="xsqb")
            nc.vector.tensor_tensor(xsqb[:], xt_b, xt_b, op=ALU.mult)

            # ---- cd2 GEMM [100,100]: shared center operands + PEN*I ----
            psum_cd2 = pp1.tile([C, C], F32, tag="cd2")
            for k in range(KD):
                nc.tensor.matmul(
                    psum_cd2[:], m2b[:, k * C : (k + 1) * C],
                    cb_ := ct_b[:, k * C : (k + 1) * C],
                    start=(k == 0), stop=False,
                )
                nc.tensor.matmul(
                    psum_cd2[:], w2b[:, k * C : (k + 1) * C],
                    csqb[:, k * C : (k + 1) * C],
                    start=False, stop=False,
                )
            nc.tensor.matmul(
                psum_cd2[:], eyepen_b[0:100, :], eye_b[0:100, :],
                start=False, stop=False,
            )
            nc.tensor.matmul(
                psum_cd2[:], arowb[:], ones_b[0:1, 0:C],
                start=False, stop=True,
            )
            cd2m = wp.tile([C, C], F32, tag="cd2m")
            nc.vector.tensor_scalar(cd2m[:], psum_cd2[:], 0.0, None, op0=ALU.max)
            cdmin2 = wp.tile([C, 1], F32, tag="cdmin2")
            nc.vector.tensor_reduce(
                cdmin2[:], cd2m[:], axis=mybir.AxisListType.X, op=ALU.min
            )
            cdminb = wp.tile([C, 1], BF16, tag="cdminb")
            nc.scalar.sqrt(cdminb[:], cdmin2[:])
            # transpose to a row, broadcast down 128 partitions
            psum_cdrow = pp1.tile([1, C], F32, tag="cdrow")
            nc.tensor.matmul(psum_cdrow[:], cdminb[:], eye_b[0:100, :])
            cdrowb = wp.tile([1, C], BF16, tag="cdrowb")
            nc.scalar.copy(cdrowb[:], psum_cdrow[:])
            psum_bc = pp1.tile([128, C], F32, tag="bcast")
            nc.tensor.matmul(psum_bc[:], ones_b[0:1, :], cdrowb[:])
            cdbb = wp.tile([128, C], BF16, tag="cdbb")
            nc.scalar.copy(cdbb[:], psum_bc[:])

            # ---- big GEMM: dc2 per 128-anchor chunk (bf16, fp32 PSUM) ----
            dc2w = wp.tile([128, MCH * C], BF16, tag="dc2w")
            for m in range(MCH):
                psum_dc2 = pp2.tile([128, C], F32, tag="dc2")
                for k in range(KD):
                    ksl = slice(XT_O + k * ROWS + m * 128,
                                XT_O + k * ROWS + (m + 1) * 128)
                    nc.tensor.matmul(
                        psum_dc2[:], xsqb[:, k * ROWS + m * 128 :
                                          k * ROWS + (m + 1) * 128],
                        w2b[:, k * C : (k + 1) * C],
                        start=(k == 0), stop=False,
                    )
                    nc.tensor.matmul(
                        psum_dc2[:], bsb[:, ksl],
                        m2b[:, k * C : (k + 1) * C],
                        start=False, stop=False,
                    )
                nc.tensor.matmul(
                    psum_dc2[:], ones_b[0:1, :], arowb[:],
                    start=False, stop=True,
                )
                nc.scalar.copy(dc2w[:, m * C : (m + 1) * C], psum_dc2[:])

            # ---- batched mining over [128, MCH, C] bf16 views ----
            dc3 = dc2w[:].rearrange("p (m c) -> p m c", c=C)
            iota3 = iota_b[:, None, :].broadcast_to([128, MCH, C])
            t3 = t_b[:, :, None].broadcast_to([128, MCH, C])
            cdb3 = cdbb[:, None, :].broadcast_to([128, MCH, C])

            ohw = kp.tile([128, MCH * C], BF16, tag="ohw")
            oh3 = ohw[:].rearrange("p (m c) -> p m c", c=C)
            nc.vector.tensor_tensor(oh3, iota3, t3, op=ALU.is_equal)
            ohpw = kp.tile([128, MCH * C], BF16, tag="ohpw")
            nc.gpsimd.tensor_scalar(ohpw[:], ohw[:], PEN, None, op0=ALU.mult)
            antw = kp.tile([128, MCH * C], BF16, tag="antw")
            nc.gpsimd.tensor_tensor(
                antw[:].rearrange("p (m c) -> p m c", c=C), dc3,
                ohpw[:].rearrange("p (m c) -> p m c", c=C), op=ALU.add
            )
            an2 = kp.tile([128, MCH], F32, tag="an2")
            nc.vector.tensor_reduce(
                an2[:], antw[:].rearrange("p (m c) -> p m c", c=C),
                axis=mybir.AxisListType.X, op=ALU.min,
            )
            aptw = kp.tile([128, MCH * C], BF16, tag="aptw")
            nc.vector.tensor_tensor(
                aptw[:].rearrange("p (m c) -> p m c", c=C), dc3, oh3,
                op=ALU.mult,
            )
            ap2 = kp.tile([128, MCH], F32, tag="ap2")
            nc.vector.tensor_reduce(
                ap2[:], aptw[:].rearrange("p (m c) -> p m c", c=C),
                axis=mybir.AxisListType.X, op=ALU.add,
            )
            cctw = kp.tile([128, MCH * C], BF16, tag="cctw")
            nc.vector.tensor_tensor(
                cctw[:].rearrange("p (m c) -> p m c", c=C), cdb3, oh3,
                op=ALU.mult,
            )
            cc4 = kp.tile([128, MCH], F32, tag="cc4")
            nc.vector.tensor_reduce(
                cc4[:], cctw[:].rearrange("p (m c) -> p m c", c=C),
                axis=mybir.AxisListType.X, op=ALU.add,
            )

            # ---- loss_i = sqrt(ap2) + relu(cc - sqrt(an2)) ----
            an = kp.tile([128, MCH], F32, tag="an")
            nc.scalar.sqrt(an[:], an2[:])
            ap = kp.tile([128, MCH], F32, tag="ap")
            nc.scalar.sqrt(ap[:], ap2[:])
            mrgin = kp.tile([128, MCH], F32, tag="mrgin")
            nc.vector.tensor_tensor(mrgin[:], cc4[:], an[:], op=ALU.subtract)
            mrg = kp.tile([128, MCH], F32, tag="mrg")
            nc.scalar.activation(mrg[:], mrgin[:], AF.Relu)
            loss4 = kp.tile([128, MCH], F32, tag="loss4")
            nc.vector.tensor_tensor(loss4[:], mrg[:], ap[:], op=ALU.add)
            losscol = kp.tile([128, 1], F32, tag="losscol")
            nc.vector.tensor_reduce(
                losscol[:], loss4[:], axis=mybir.AxisListType.X, op=ALU.add
            )

            psum_loss = pp1.tile([1, 1], F32, tag="loss")
            nc.tensor.matmul(psum_loss[:], ones_f[:, 0:1], losscol[:])
            res_sb = wp.tile([1, 1], F32, tag="res")
            nc.scalar.copy(res_sb[:], psum_loss[:])
            nc.sync.dma_start(out[:], res_sb[:])

    nc.compile()
    return nc


_NC_CACHE: list = []


def _get_nc() -> bass.Bass:
    if not _NC_CACHE:
        _NC_CACHE.append(build_nc())
    return _NC_CACHE[0]


def make_in_maps(inputs, centers, centers_weights, targets):
    x = np.asarray(inputs, dtype=np.float32)
    c = np.asarray(centers, dtype=np.float32)
    cw = np.asarray(centers_weights, dtype=np.float32)
    t = np.asarray(targets).astype(np.int64)

    bconst = np.zeros((128, BW), dtype=np.float32)
    bconst[:, IOTA_O : IOTA_O + 100] = np.arange(C, dtype=np.float32)[None, :]
    eye = np.eye(C, dtype=np.float32)
    bconst[0:100, BEYE_O : BEYE_O + 100] = eye
    bconst[0:100, BPEN_O : BPEN_O + 100] = PEN * eye
    bconst[:, BONES_O : BONES_O + 128] = 1.0
    cT = c.T.reshape(KD, 128, C)
    cwT = cw.T.reshape(KD, 128, C)
    for k in range(KD):
        bconst[:, CT_O + k * C : CT_O + (k + 1) * C] = cT[k]
        bconst[:, CWT_O + k * C : CWT_O + (k + 1) * C] = cwT[k]

    fshared = np.zeros((128, FW), dtype=np.float32)
    fshared[:, 0:1] = 1.0
    present = np.zeros(C, dtype=bool)
    present[np.unique(t)] = True
    fshared[0, FPEN_O : FPEN_O + C] = np.where(present, 0.0, PEN)

    xT = np.ascontiguousarray(x.T)                      # [D, B]

    in_maps = []
    for i in range(NCORES):
        rows = slice(i * ROWS, (i + 1) * ROWS)
        bcst = bconst.copy()
        xs = xT[:, rows].reshape(KD, 128, ROWS)
        for k in range(KD):
            bcst[:, XT_O + k * ROWS : XT_O + (k + 1) * ROWS] = xs[k]
        ts = t[rows].astype(np.float32).reshape(MCH, 128)
        bcst[:, T_O : T_O + MCH] = ts.T
        in_maps.append({
            "bin": bcst.astype(ml_dtypes.bfloat16),
            "fin": fshared,
        })
    return in_maps


def kernel(inputs, centers, centers_weights, targets, epoch_number=None,
           **_ignored):
    nc = _get_nc()
    in_maps = make_in_maps(inputs, centers, centers_weights, targets)
    res = run_bass_kernel_spmd(nc, in_maps, core_ids=list(range(NCORES)))
    total = sum(float(r["out"][0, 0]) for r in res.results)
    return np.float32(total / B)


# revision 21
# speedup vs baseline: 1.5454x; 1.0277x over previous
"""Trainium2 Bass kernel for the AMTCL loss (nn_AMTCL_66520453480770).

Math: the reference builds a [B,B] pairwise distance matrix dist[i,j] between
inputs[i] and centers[targets[j]] (weights 2**centers_weights[targets[j]]).
Since dist[i,j] depends on j only through c = targets[j], the whole problem
collapses to the [B,C] matrix
    dc2[i,c] = sum_d w2[c,d] * (centers[c,d] - inputs[i,d])**2
with
    dist_ap[i] = sqrt(dc2[i, t_i])            (all same-class j are equal)
    dist_an[i] = sqrt(min_{c present, c != t_i} dc2[i,c])   (sqrt commutes
                 with min, so mining happens on squared distances)
    cc[i]      = centers_dist[t_i],  centers_dist[c] = sqrt(min_{j!=c} cd2[c,j])
    loss       = mean(dist_ap + relu(cc - dist_an))
This is exactly equal to the reference (40x less compute than the B^2 form);
compute runs in bf16 with fp32 PSUM accumulation (loss rel err ~1e-4).

dc2 is one GEMM with contraction K = 2D+1:
    dc2[i,c] = sum_d xsq[i,d]*w2[c,d] + sum_d x[i,d]*(-2*w2[c,d]*c[c,d]) + a[c]
The a[c] row rides in as a K=1 term; the cd2 GEMM shares the same center-side
operands and adds 2^40*I via an identity matmul to mask the diagonal.

Sharding: data-parallel over the 4096 anchor rows -> 8 cores x 512 rows.
centers/centers_weights replicated. Each core emits a partial loss sum [1,1];
the host sums the 8 scalars and divides by B.
"""

import ml_dtypes
import numpy as np

import concourse.bass as bass
import concourse.bacc as bacc
import concourse.mybir as mybir
import concourse.tile as tile
from concourse.bass_utils import run_bass_kernel_spmd

B, C, D = 4096, 100, 384
NCORES = 8
ROWS = B // NCORES          # 512 anchor rows per core
MCH = ROWS // 128           # 4 partition chunks of anchor rows
KD = D // 128               # 3 partition chunks of the feature dim
PEN = float(2 ** 40)        # self-class / absent-class / diagonal penalty
LN2 = float(np.log(2.0))
F32 = mybir.dt.float32
BF16 = mybir.dt.bfloat16
AF = mybir.ActivationFunctionType
ALU = mybir.AluOpType

# bf16 const block layout (columns)
IOTA_O = 0                   # iota row 0..99, all partitions
T_O = IOTA_O + 100           # targets column-chunks [128, MCH]
BEYE_O = T_O + MCH           # eye(100)
BPEN_O = BEYE_O + 100        # PEN * eye(100)
BONES_O = BPEN_O + 100       # all-ones [128,128]
CT_O = BONES_O + 128         # centers.T chunks [128, 3*100]
CWT_O = CT_O + KD * C        # centers_weights.T chunks
XT_O = CWT_O + KD * C        # x.T shard chunks [128, 3*512]
BW = XT_O + KD * ROWS        # 2568

# f32 const block: ones column | absent-class penalty row (row 0)
FPEN_O = 1
FW = FPEN_O + C


def build_nc() -> bass.Bass:
    nc = bacc.Bacc(
        "TRN2", target_bir_lowering=False, debug=False, num_devices=NCORES
    )

    bin_ = nc.declare_dram_parameter("bin", [128, BW], BF16, isOutput=False)
    fin = nc.declare_dram_parameter("fin", [128, FW], F32, isOutput=False)
    out = nc.declare_dram_parameter("out", [1, 1], F32, isOutput=True)

    with tile.TileContext(nc) as tc:
        with (
            tc.tile_pool(name="wts", bufs=1) as wp,
            tc.tile_pool(name="work", bufs=2) as kp,
            tc.tile_pool(name="ps1", bufs=1, space="PSUM") as pp1,
            tc.tile_pool(name="ps2", bufs=2, space="PSUM") as pp2,
        ):
            # consts+centers land first (unblock prologue); x separately
            bsb = wp.tile([128, XT_O], BF16, tag="bsb")
            nc.sync.dma_start(bsb[:], bin_[:, 0:XT_O])
            xtile = wp.tile([128, KD * ROWS], BF16, tag="xtile")
            nc.sync.dma_start(xtile[:], bin_[:, XT_O:BW])
            fsb = wp.tile([128, FW], F32, tag="fsb")
            nc.sync.dma_start(fsb[:], fin[:])

            iota_b = bsb[:, IOTA_O : IOTA_O + 100]
            t_b = bsb[:, T_O : T_O + MCH]
            eye_b = bsb[:, BEYE_O : BEYE_O + 100]
            eyepen_b = bsb[:, BPEN_O : BPEN_O + 100]
            ones_b = bsb[:, BONES_O : BONES_O + 128]
            ct_b = bsb[:, CT_O : CT_O + KD * C]
            cwt_b = bsb[:, CWT_O : CWT_O + KD * C]
            xt_b = xtile[:]
            ones_f = fsb[:, 0:1]
            penrow_f = fsb[0:1, FPEN_O : FPEN_O + C]

            # ---- center-side prep (all bf16, fast engines) ----
            w2b = wp.tile([128, KD * C], BF16, tag="w2b")
            nc.scalar.activation(w2b[:], cwt_b, AF.Exp, scale=LN2)
            # dummy sqrt: pulls the sqrt ACT table load off the critical
            # path; reads w2b so it lands after the Exp (exp-table) use.
            sqdummy = wp.tile([1, 1], F32, tag="sqdummy")
            nc.scalar.sqrt(sqdummy[:], w2b[0:1, 0:1])

            csqb = wp.tile([128, KD * C], BF16, tag="csqb")
            nc.scalar.square(csqb[:], ct_b)
            cm2b = wp.tile([128, KD * C], BF16, tag="cm2b")
            nc.vector.tensor_scalar(cm2b[:], ct_b, -2.0, None, op0=ALU.mult)
            m2b = wp.tile([128, KD * C], BF16, tag="m2b")
            nc.vector.tensor_tensor(m2b[:], w2b[:], cm2b[:], op=ALU.mult)
            wsqb = wp.tile([128, KD * C], BF16, tag="wsqb")
            nc.vector.tensor_tensor(wsqb[:], w2b[:], csqb[:], op=ALU.mult)

            # a[c] = sum_d w2*c^2, + absent-class penalty row -> bf16
            psum_arow = pp1.tile([1, C], F32, tag="arow")
            for k in range(KD):
                nc.tensor.matmul(
                    psum_arow[:], ones_b[:, 0:1],
                    wsqb[:, k * C : (k + 1) * C],
                    start=(k == 0), stop=(k == KD - 1),
                )
            arowb = wp.tile([1, C], BF16, tag="arowb")
            nc.vector.tensor_tensor(
                arowb[:], psum_arow[:], penrow_f, op=ALU.add
            )

            # ---- anchor-side prep ----
            xsqb = wp.tile([128, KD * ROWS], BF16, tag="xsqb")
            nc.scalar.square(xsqb[:], xt_b)

            # ---- cd2 GEMM [100,100]: shared center operands + PEN*I ----
            psum_cd2 = pp1.tile([C, C], F32, tag="cd2")
            for k in range(KD):
                nc.tensor.matmul(
                    psum_cd2[:], m2b[:, k * C : (k + 1) * C],
                    cb_ := ct_b[:, k * C : (k + 1) * C],
                    start=(k == 0), stop=False,
                )
                nc.tensor.matmul(
                    psum_cd2[:], w2b[:, k * C : (k + 1) * C],
                    csqb[:, k * C : (k + 1) * C],
                    start=False, stop=False,
                )
            nc.tensor.matmul(
                psum_cd2[:], eyepen_b[0:100, :], eye_b[0:100, :],
                start=False, stop=False,
            )
            nc.tensor.matmul(
                psum_cd2[:], arowb[:], ones_b[0:1, 0:C],
                start=False, stop=True,
            )
            cd2m = wp.tile([C, C], F32, tag="cd2m")
            nc.vector.tensor_scalar(cd2m[:], psum_cd2[:], 0.0, None, op0=ALU.max)
            cdmin2 = wp.tile([C, 1], F32, tag="cdmin2")
            nc.vector.tensor_reduce(
                cdmin2[:], cd2m[:], axis=mybir.AxisListType.X, op=ALU.min
            )
            cdminb = wp.tile([C, 1], BF16, tag="cdminb")
            nc.scalar.sqrt(cdminb[:], cdmin2[:])
            # transpose to a row, broadcast down 128 partitions
            psum_cdrow = pp1.tile([1, C], F32, tag="cdrow")
            nc.tensor.matmul(psum_cdrow[:], cdminb[:], eye_b[0:100, :])
            cdrowb = wp.tile([1, C], BF16, tag="cdrowb")
            nc.scalar.copy(cdrowb[:], psum_cdrow[:])
            psum_bc = pp1.tile([128, C], F32, tag="bcast")
            nc.tensor.matmul(psum_bc[:], ones_b[0:1, :], cdrowb[:])
            cdbb = wp.tile([128, C], BF16, tag="cdbb")
            nc.scalar.copy(cdbb[:], psum_bc[:])

            # ---- big GEMM: dc2 per 128-anchor chunk (bf16, fp32 PSUM) ----
            dc2w = wp.tile([128, MCH * C], BF16, tag="dc2w")
            for m in range(MCH):
                psum_dc2 = pp2.tile([128, C], F32, tag="dc2")
                for k in range(KD):
                    nc.tensor.matmul(
                        psum_dc2[:], xsqb[:, k * ROWS + m * 128 :
                                          k * ROWS + (m + 1) * 128],
                        w2b[:, k * C : (k + 1) * C],
                        start=(k == 0), stop=False,
                    )
                    nc.tensor.matmul(
                        psum_dc2[:], xt_b[:, k * ROWS + m * 128 :
                                          k * ROWS + (m + 1) * 128],
                        m2b[:, k * C : (k + 1) * C],
                        start=False, stop=False,
                    )
                nc.tensor.matmul(
                    psum_dc2[:], ones_b[0:1, :], arowb[:],
                    start=False, stop=True,
                )
                nc.scalar.copy(dc2w[:, m * C : (m + 1) * C], psum_dc2[:])

            # ---- batched mining over [128, MCH, C] bf16 views ----
            dc3 = dc2w[:].rearrange("p (m c) -> p m c", c=C)
            iota3 = iota_b[:, None, :].broadcast_to([128, MCH, C])
            t3 = t_b[:, :, None].broadcast_to([128, MCH, C])
            cdb3 = cdbb[:, None, :].broadcast_to([128, MCH, C])

            ohw = kp.tile([128, MCH * C], BF16, tag="ohw")
            oh3 = ohw[:].rearrange("p (m c) -> p m c", c=C)
            nc.vector.tensor_tensor(oh3, iota3, t3, op=ALU.is_equal)
            ohpw = kp.tile([128, MCH * C], BF16, tag="ohpw")
            nc.vector.tensor_scalar(ohpw[:], ohw[:], PEN, None, op0=ALU.mult)
            antw = kp.tile([128, MCH * C], BF16, tag="antw")
            nc.vector.tensor_tensor(
                antw[:].rearrange("p (m c) -> p m c", c=C), dc3,
                ohpw[:].rearrange("p (m c) -> p m c", c=C), op=ALU.add
            )
            an2 = kp.tile([128, MCH], F32, tag="an2")
            nc.vector.tensor_reduce(
                an2[:], antw[:].rearrange("p (m c) -> p m c", c=C),
                axis=mybir.AxisListType.X, op=ALU.min,
            )
            aptw = kp.tile([128, MCH * C], BF16, tag="aptw")
            nc.vector.tensor_tensor(
                aptw[:].rearrange("p (m c) -> p m c", c=C), dc3, oh3,
                op=ALU.mult,
            )
            ap2 = kp.tile([128, MCH], F32, tag="ap2")
            nc.vector.tensor_reduce(
                ap2[:], aptw[:].rearrange("p (m c) -> p m c", c=C),
                axis=mybir.AxisListType.X, op=ALU.add,
            )
            cctw = kp.tile([128, MCH * C], BF16, tag="cctw")
            nc.vector.tensor_tensor(
                cctw[:].rearrange("p (m c) -> p m c", c=C), cdb3, oh3,
                op=ALU.mult,
            )
            cc4 = kp.tile([128, MCH], F32, tag="cc4")
            nc.vector.tensor_reduce(
                cc4[:], cctw[:].rearrange("p (m c) -> p m c", c=C),
                axis=mybir.AxisListType.X, op=ALU.add,
            )

            # ---- loss_i = sqrt(ap2) + relu(cc - sqrt(an2)) ----
            an = kp.tile([128, MCH], F32, tag="an")
            nc.scalar.sqrt(an[:], an2[:])
            ap = kp.tile([128, MCH], F32, tag="ap")
            nc.scalar.sqrt(ap[:], ap2[:])
            mrgin = kp.tile([128, MCH], F32, tag="mrgin")
            nc.vector.tensor_tensor(mrgin[:], cc4[:], an[:], op=ALU.subtract)
            mrg = kp.tile([128, MCH], F32, tag="mrg")
            nc.scalar.activation(mrg[:], mrgin[:], AF.Relu)
            loss4 = kp.tile([128, MCH], F32, tag="loss4")
            nc.vector.tensor_tensor(loss4[:], mrg[:], ap[:], op=ALU.add)
            losscol = kp.tile([128, 1], F32, tag="losscol")
            nc.vector.tensor_reduce(
                losscol[:], loss4[:], axis=mybir.AxisListType.X, op=ALU.add
            )

            psum_loss = pp1.tile([1, 1], F32, tag="loss")
            nc.tensor.matmul(psum_loss[:], ones_f[:, 0:1], losscol[:])
            res_sb = wp.tile([1, 1], F32, tag="res")
            nc.scalar.copy(res_sb[:], psum_loss[:])
            nc.sync.dma_start(out[:], res_sb[:])

    nc.compile()
    return nc


_NC_CACHE: list = []


def _get_nc() -> bass.Bass:
    if not _NC_CACHE:
        _NC_CACHE.append(build_nc())
    return _NC_CACHE[0]


def make_in_maps(inputs, centers, centers_weights, targets):
    x = np.asarray(inputs, dtype=np.float32)
    c = np.asarray(centers, dtype=np.float32)
    cw = np.asarray(centers_weights, dtype=np.float32)
    t = np.asarray(targets).astype(np.int64)

    bconst = np.zeros((128, BW), dtype=np.float32)
    bconst[:, IOTA_O : IOTA_O + 100] = np.arange(C, dtype=np.float32)[None, :]
    eye = np.eye(C, dtype=np.float32)
    bconst[0:100, BEYE_O : BEYE_O + 100] = eye
    bconst[0:100, BPEN_O : BPEN_O + 100] = PEN * eye
    bconst[:, BONES_O : BONES_O + 128] = 1.0
    cT = c.T.reshape(KD, 128, C)
    cwT = cw.T.reshape(KD, 128, C)
    for k in range(KD):
        bconst[:, CT_O + k * C : CT_O + (k + 1) * C] = cT[k]
        bconst[:, CWT_O + k * C : CWT_O + (k + 1) * C] = cwT[k]

    fshared = np.zeros((128, FW), dtype=np.float32)
    fshared[:, 0:1] = 1.0
    present = np.zeros(C, dtype=bool)
    present[np.unique(t)] = True
    fshared[0, FPEN_O : FPEN_O + C] = np.where(present, 0.0, PEN)

    xT = np.ascontiguousarray(x.T)                      # [D, B]

    in_maps = []
    for i in range(NCORES):
        rows = slice(i * ROWS, (i + 1) * ROWS)
        bcst = bconst.copy()
        xs = xT[:, rows].reshape(KD, 128, ROWS)
        for k in range(KD):
            bcst[:, XT_O + k * ROWS : XT_O + (k + 1) * ROWS] = xs[k]
        ts = t[rows].astype(np.float32).reshape(MCH, 128)
        bcst[:, T_O : T_O + MCH] = ts.T
        in_maps.append({
            "bin": bcst.astype(ml_dtypes.bfloat16),
            "fin": fshared,
        })
    return in_maps


def kernel(inputs, centers, centers_weights, targets, epoch_number=None,
           **_ignored):
    nc = _get_nc()
    in_maps = make_in_maps(inputs, centers, centers_weights, targets)
    res = run_bass_kernel_spmd(nc, in_maps, core_ids=list(range(NCORES)))
    total = sum(float(r["out"][0, 0]) for r in res.results)
    return np.float32(total / B)


# revision 22
# speedup vs baseline: 1.7768x; 1.1498x over previous
"""Trainium2 Bass kernel for the AMTCL loss (nn_AMTCL_66520453480770).

Math: the reference builds a [B,B] pairwise distance matrix dist[i,j] between
inputs[i] and centers[targets[j]] (weights 2**centers_weights[targets[j]]).
Since dist[i,j] depends on j only through c = targets[j], the whole problem
collapses to the [B,C] matrix
    dc2[i,c] = sum_d w2[c,d] * (centers[c,d] - inputs[i,d])**2
with
    dist_ap[i] = sqrt(dc2[i, t_i])            (all same-class j are equal)
    dist_an[i] = sqrt(min_{c present, c != t_i} dc2[i,c])   (sqrt commutes
                 with min, so mining happens on squared distances)
    cc[i]      = centers_dist[t_i],  centers_dist[c] = sqrt(min_{j!=c} cd2[c,j])
    loss       = mean(dist_ap + relu(cc - dist_an))
This is exactly equal to the reference (40x less compute than the B^2 form);
GEMMs run in bf16 with fp32 PSUM accumulation (loss rel err ~1e-4).

dc2 is one GEMM with contraction K = 2D+1:
    dc2[i,c] = sum_d xsq[i,d]*w2[c,d] + sum_d x[i,d]*(-2*w2[c,d]*c[c,d]) + a[c]
The a[c] row rides in as a K=1 term; the cd2 GEMM shares the same center-side
operands and adds 2^40*I via an identity matmul to mask the diagonal.
Mining reads dc2 straight out of PSUM per 128-anchor chunk, overlapped with
the next chunk's matmuls.

Sharding: data-parallel over the 4096 anchor rows -> 8 cores x 512 rows.
centers/centers_weights replicated. Each core emits a partial loss sum [1,1];
the host sums the 8 scalars and divides by B.
"""

import ml_dtypes
import numpy as np

import concourse.bass as bass
import concourse.bacc as bacc
import concourse.mybir as mybir
import concourse.tile as tile
from concourse.bass_utils import run_bass_kernel_spmd

B, C, D = 4096, 100, 384
NCORES = 8
ROWS = B // NCORES          # 512 anchor rows per core
MCH = ROWS // 128           # 4 partition chunks of anchor rows
KD = D // 128               # 3 partition chunks of the feature dim
PEN = float(2 ** 40)        # self-class / absent-class / diagonal penalty
LN2 = float(np.log(2.0))
F32 = mybir.dt.float32
BF16 = mybir.dt.bfloat16
AF = mybir.ActivationFunctionType
ALU = mybir.AluOpType

# bf16 const block layout (columns)
IOTA_O = 0                   # iota row 0..99, all partitions
T_O = IOTA_O + 100           # targets column-chunks [128, MCH]
BEYE_O = T_O + MCH           # eye(100)
BPEN_O = BEYE_O + 100        # PEN * eye(100)
BONES_O = BPEN_O + 100       # all-ones [128,128]
CT_O = BONES_O + 128         # centers.T chunks [128, 3*100]
CWT_O = CT_O + KD * C        # centers_weights.T chunks
XT_O = CWT_O + KD * C        # x.T shard chunks [128, 3*512]
BW = XT_O + KD * ROWS        # 2568

# f32 const block: ones column | absent-class penalty row (row 0)
FPEN_O = 1
FW = FPEN_O + C


def build_nc() -> bass.Bass:
    nc = bacc.Bacc(
        "TRN2", target_bir_lowering=False, debug=False, num_devices=NCORES
    )

    bin_ = nc.declare_dram_parameter("bin", [128, BW], BF16, isOutput=False)
    fin = nc.declare_dram_parameter("fin", [128, FW], F32, isOutput=False)
    out = nc.declare_dram_parameter("out", [1, 1], F32, isOutput=True)

    with tile.TileContext(nc) as tc:
        with (
            tc.tile_pool(name="wts", bufs=1) as wp,
            tc.tile_pool(name="work", bufs=2) as kp,
            tc.tile_pool(name="ps1", bufs=1, space="PSUM") as pp1,
            tc.tile_pool(name="ps2", bufs=2, space="PSUM") as pp2,
        ):
            # consts+centers land first (unblock prologue); x in 3 chunks
            bsb = wp.tile([128, XT_O], BF16, tag="bsb")
            nc.sync.dma_start(bsb[:], bin_[:, 0:XT_O])
            xtiles = []
            for k in range(KD):
                xk = wp.tile([128, ROWS], BF16, tag=f"xt{k}")
                nc.sync.dma_start(
                    xk[:], bin_[:, XT_O + k * ROWS : XT_O + (k + 1) * ROWS]
                )
                xtiles.append(xk)
            fsb = wp.tile([128, FW], F32, tag="fsb")
            nc.sync.dma_start(fsb[:], fin[:])

            iota_b = bsb[:, IOTA_O : IOTA_O + 100]
            t_b = bsb[:, T_O : T_O + MCH]
            eye_b = bsb[:, BEYE_O : BEYE_O + 100]
            eyepen_b = bsb[:, BPEN_O : BPEN_O + 100]
            ones_b = bsb[:, BONES_O : BONES_O + 128]
            ct_b = bsb[:, CT_O : CT_O + KD * C]
            cwt_b = bsb[:, CWT_O : CWT_O + KD * C]
            ones_f = fsb[:, 0:1]
            penrow_f = fsb[0:1, FPEN_O : FPEN_O + C]

            # ---- center-side prep (bf16) ----
            w2b = wp.tile([128, KD * C], BF16, tag="w2b")
            nc.scalar.activation(w2b[:], cwt_b, AF.Exp, scale=LN2)
            # dummy sqrt: pulls the sqrt ACT table load off the critical
            # path; reads w2b so it lands after the Exp (exp-table) use.
            sqdummy = wp.tile([1, 1], F32, tag="sqdummy")
            nc.scalar.sqrt(sqdummy[:], w2b[0:1, 0:1])

            csqb = wp.tile([128, KD * C], BF16, tag="csqb")
            nc.scalar.square(csqb[:], ct_b)
            cm2b = wp.tile([128, KD * C], BF16, tag="cm2b")
            nc.vector.tensor_scalar(cm2b[:], ct_b, -2.0, None, op0=ALU.mult)
            m2b = wp.tile([128, KD * C], BF16, tag="m2b")
            nc.vector.tensor_tensor(m2b[:], w2b[:], cm2b[:], op=ALU.mult)
            wsqb = wp.tile([128, KD * C], BF16, tag="wsqb")
            nc.vector.tensor_tensor(wsqb[:], w2b[:], csqb[:], op=ALU.mult)

            # one-hot masks (only need iota/t -> very early)
            ohw = wp.tile([128, MCH * C], F32, tag="ohw")
            oh3 = ohw[:].rearrange("p (m c) -> p m c", c=C)
            nc.vector.tensor_tensor(
                oh3, iota_b[:, None, :].broadcast_to([128, MCH, C]),
                t_b[:, :, None].broadcast_to([128, MCH, C]), op=ALU.is_equal
            )
            ohpw = wp.tile([128, MCH * C], F32, tag="ohpw")
            nc.vector.tensor_scalar(ohpw[:], ohw[:], PEN, None, op0=ALU.mult)

            # a[c] = sum_d w2*c^2, + absent-class penalty row -> bf16
            psum_arow = pp1.tile([1, C], F32, tag="arow")
            for k in range(KD):
                nc.tensor.matmul(
                    psum_arow[:], ones_b[:, 0:1],
                    wsqb[:, k * C : (k + 1) * C],
                    start=(k == 0), stop=(k == KD - 1),
                )
            arowb = wp.tile([1, C], BF16, tag="arowb")
            nc.vector.tensor_tensor(
                arowb[:], psum_arow[:], penrow_f, op=ALU.add
            )

            # x^2 per k-chunk (pipelines with the x DMA chunks)
            xsqtiles = []
            for k in range(KD):
                xsq = wp.tile([128, ROWS], BF16, tag=f"xsq{k}")
                nc.scalar.square(xsq[:], xtiles[k][:])
                xsqtiles.append(xsq)

            # ---- cd2 GEMM [100,100]: shared center operands + PEN*I ----
            psum_cd2 = pp1.tile([C, C], F32, tag="cd2")
            for k in range(KD):
                nc.tensor.matmul(
                    psum_cd2[:], m2b[:, k * C : (k + 1) * C],
                    ct_b[:, k * C : (k + 1) * C],
                    start=(k == 0), stop=False,
                )
                nc.tensor.matmul(
                    psum_cd2[:], w2b[:, k * C : (k + 1) * C],
                    csqb[:, k * C : (k + 1) * C],
                    start=False, stop=False,
                )
            nc.tensor.matmul(
                psum_cd2[:], eyepen_b[0:100, :], eye_b[0:100, :],
                start=False, stop=False,
            )
            nc.tensor.matmul(
                psum_cd2[:], arowb[:], ones_b[0:1, 0:C],
                start=False, stop=True,
            )
            cd2m = wp.tile([C, C], F32, tag="cd2m")
            nc.vector.tensor_scalar(cd2m[:], psum_cd2[:], 0.0, None, op0=ALU.max)
            cdmin2 = wp.tile([C, 1], F32, tag="cdmin2")
            nc.vector.tensor_reduce(
                cdmin2[:], cd2m[:], axis=mybir.AxisListType.X, op=ALU.min
            )
            cdminb = wp.tile([C, 1], BF16, tag="cdminb")
            nc.scalar.sqrt(cdminb[:], cdmin2[:])
            # transpose to a row, broadcast down 128 partitions
            psum_cdrow = pp1.tile([1, C], F32, tag="cdrow")
            nc.tensor.matmul(psum_cdrow[:], cdminb[:], eye_b[0:100, :])
            cdrowb = wp.tile([1, C], BF16, tag="cdrowb")
            nc.scalar.copy(cdrowb[:], psum_cdrow[:])
            psum_bc = pp1.tile([128, C], F32, tag="bcast")
            nc.tensor.matmul(psum_bc[:], ones_b[0:1, :], cdrowb[:])
            cdbf = wp.tile([128, C], F32, tag="cdbf")
            nc.scalar.copy(cdbf[:], psum_bc[:])

            # cc[i] = centers_dist[t_i] (dc2-independent -> overlapped)
            cctw = kp.tile([128, MCH * C], F32, tag="cctw")
            nc.vector.tensor_tensor(
                cctw[:].rearrange("p (m c) -> p m c", c=C),
                cdbf[:, None, :].broadcast_to([128, MCH, C]), oh3,
                op=ALU.mult,
            )
            cc4 = kp.tile([128, MCH], F32, tag="cc4")
            nc.vector.tensor_reduce(
                cc4[:], cctw[:].rearrange("p (m c) -> p m c", c=C),
                axis=mybir.AxisListType.X, op=ALU.add,
            )

            # ---- big GEMM + per-chunk mining straight out of PSUM ----
            an2 = kp.tile([128, MCH], F32, tag="an2")
            ap2 = kp.tile([128, MCH], F32, tag="ap2")
            for m in range(MCH):
                psum_dc2 = pp2.tile([128, C], F32, tag="dc2")
                for k in range(KD):
                    nc.tensor.matmul(
                        psum_dc2[:],
                        xtiles[k][:, m * 128 : (m + 1) * 128],
                        m2b[:, k * C : (k + 1) * C],
                        start=(k == 0), stop=False,
                    )
                for k in range(KD):
                    nc.tensor.matmul(
                        psum_dc2[:],
                        xsqtiles[k][:, m * 128 : (m + 1) * 128],
                        w2b[:, k * C : (k + 1) * C],
                        start=False, stop=False,
                    )
                nc.tensor.matmul(
                    psum_dc2[:], ones_b[0:1, :], arowb[:],
                    start=False, stop=True,
                )
                antm = kp.tile([128, C], F32, tag="antm")
                nc.vector.tensor_tensor(
                    antm[:], psum_dc2[:], ohpw[:, m * C : (m + 1) * C],
                    op=ALU.add,
                )
                nc.vector.tensor_reduce(
                    an2[:, m : m + 1], antm[:],
                    axis=mybir.AxisListType.X, op=ALU.min,
                )
                aptm = kp.tile([128, C], F32, tag="aptm")
                nc.vector.tensor_tensor(
                    aptm[:], psum_dc2[:], ohw[:, m * C : (m + 1) * C],
                    op=ALU.mult,
                )
                nc.vector.tensor_reduce(
                    ap2[:, m : m + 1], aptm[:],
                    axis=mybir.AxisListType.X, op=ALU.add,
                )

            # ---- loss_i = sqrt(ap2) + relu(cc - sqrt(an2)) ----
            an = kp.tile([128, MCH], F32, tag="an")
            nc.scalar.sqrt(an[:], an2[:])
            mrgin = kp.tile([128, MCH], F32, tag="mrgin")
            nc.vector.tensor_tensor(mrgin[:], cc4[:], an[:], op=ALU.subtract)
            # sum_m relu(margin) and sum_m sqrt(ap2) via accum_out
            mrg = kp.tile([128, MCH], F32, tag="mrg")
            relusum = kp.tile([128, 1], F32, tag="relusum")
            nc.scalar.activation(mrg[:], mrgin[:], AF.Relu,
                                 accum_out=relusum[:])
            ap = kp.tile([128, MCH], F32, tag="ap")
            apsum = kp.tile([128, 1], F32, tag="apsum")
            nc.scalar.activation(ap[:], ap2[:], AF.Sqrt, accum_out=apsum[:])
            losscol = kp.tile([128, 1], F32, tag="losscol")
            nc.vector.tensor_tensor(
                losscol[:], relusum[:], apsum[:], op=ALU.add
            )

            psum_loss = pp1.tile([1, 1], F32, tag="loss")
            nc.tensor.matmul(psum_loss[:], ones_f[:, 0:1], losscol[:])
            res_sb = wp.tile([1, 1], F32, tag="res")
            nc.scalar.copy(res_sb[:], psum_loss[:])
            nc.sync.dma_start(out[:], res_sb[:])

    nc.compile()
    return nc


_NC_CACHE: list = []


def _get_nc() -> bass.Bass:
    if not _NC_CACHE:
        _NC_CACHE.append(build_nc())
    return _NC_CACHE[0]


def make_in_maps(inputs, centers, centers_weights, targets):
    x = np.asarray(inputs, dtype=np.float32)
    c = np.asarray(centers, dtype=np.float32)
    cw = np.asarray(centers_weights, dtype=np.float32)
    t = np.asarray(targets).astype(np.int64)

    bconst = np.zeros((128, BW), dtype=np.float32)
    bconst[:, IOTA_O : IOTA_O + 100] = np.arange(C, dtype=np.float32)[None, :]
    eye = np.eye(C, dtype=np.float32)
    bconst[0:100, BEYE_O : BEYE_O + 100] = eye
    bconst[0:100, BPEN_O : BPEN_O + 100] = PEN * eye
    bconst[:, BONES_O : BONES_O + 128] = 1.0
    cT = c.T.reshape(KD, 128, C)
    cwT = cw.T.reshape(KD, 128, C)
    for k in range(KD):
        bconst[:, CT_O + k * C : CT_O + (k + 1) * C] = cT[k]
        bconst[:, CWT_O + k * C : CWT_O + (k + 1) * C] = cwT[k]

    fshared = np.zeros((128, FW), dtype=np.float32)
    fshared[:, 0:1] = 1.0
    present = np.zeros(C, dtype=bool)
    present[np.unique(t)] = True
    fshared[0, FPEN_O : FPEN_O + C] = np.where(present, 0.0, PEN)

    xT = np.ascontiguousarray(x.T)                      # [D, B]

    in_maps = []
    for i in range(NCORES):
        rows = slice(i * ROWS, (i + 1) * ROWS)
        bcst = bconst.copy()
        xs = xT[:, rows].reshape(KD, 128, ROWS)
        for k in range(KD):
            bcst[:, XT_O + k * ROWS : XT_O + (k + 1) * ROWS] = xs[k]
        ts = t[rows].astype(np.float32).reshape(MCH, 128)
        bcst[:, T_O : T_O + MCH] = ts.T
        in_maps.append({
            "bin": bcst.astype(ml_dtypes.bfloat16),
            "fin": fshared,
        })
    return in_maps


def kernel(inputs, centers, centers_weights, targets, epoch_number=None,
           **_ignored):
    nc = _get_nc()
    in_maps = make_in_maps(inputs, centers, centers_weights, targets)
    res = run_bass_kernel_spmd(nc, in_maps, core_ids=list(range(NCORES)))
    total = sum(float(r["out"][0, 0]) for r in res.results)
    return np.float32(total / B)
